# revision 1
# baseline (speedup 1.0000x reference)
"""Trainium2 Bass kernel for nn_Classifier (GNN edge classifier).

Reference computation, per edge e with src s=idx[0,e], dst d=idx[1,e]:
    out[e] = W2 @ relu(W1 @ [x_disease[s]; x_drug[d]] + b1) + b2

Algebraic restructure: W1 = [W1a | W1b] (each [256,128]), so
    h = relu(A[s] + B[d])  with  A = x_disease @ W1a.T + b1,  B = x_drug @ W1b.T
    out[e] = W2 @ h + b2

Per-core plan (8-way data parallel over edges, ~125k edges/core):
  Phase A/B (TileContext): load consts; compute A,B ([n_nodes,256] bf16) on
    device (PE matmuls) from host-transposed x tables; store to DRAM.
  Phase C (raw engine blocks; the Tile exit drain is the phase barrier):
    per block of 4096 edges: 4+4 dma_gather (1024 idx each — the
    65-descriptor single-packet HW limit) from A/B spread over 4 SWDGE
    queues (each queue owns its own completion sems; ~2.4us/gather
    measured) into [128, 32, 256] tiles (edge q=g*128+p at [p, g, :]);
    DVE: add, fused (max 0)*w2, bf16 tree-reduce 256->64, f32 reduce,
    +b2, 32x32 block transposes; Sync DMAs rows out.
"""

import sys
import types
from contextlib import ExitStack

import numpy as np

import concourse.bacc as bacc
import concourse.bass as bass
import concourse.mybir as mybir
import concourse.tile as tile
from concourse.library_config import mlp as _mlp_lib

F32 = mybir.dt.float32
BF16 = mybir.dt.bfloat16
I16 = mybir.dt.int16


def _cdiv(a, b):
    return (a + b - 1) // b


class Cfg:
    def __init__(self, n_nodes=20000, e_core=125000, node_chunk=1024):
        self.n_nodes = n_nodes
        self.e_core = e_core
        self.gi = 1024                       # idx per dma_gather (HW packet limit)
        self.gpb = 4                         # gathers per block per table
        self.epb = self.gi * self.gpb        # 4096 edges per block
        self.nblk = _cdiv(e_core, self.epb)
        self.e_pad = self.nblk * self.epb
        self.ngrp = self.epb // 128          # 32 groups per block
        self.idx_cols = self.e_pad // 16     # wrapped idx columns per table
        self.node_chunk = node_chunk
        self.n_rows = _cdiv(n_nodes, 128) * 128


FULL = Cfg()
N_CORES = 8
E_TOTAL = 1_000_000
NQ = 4  # SWDGE queues


def build(nc, io, cfg):
    """Emit the per-core program (Tile phases A/B + raw phase C)."""
    c = cfg
    stack = ExitStack()
    with stack:
        # ---- raw SBUF state shared with phase C (allocated first) ----
        w2_b = stack.enter_context(nc.sbuf_tensor("w2_b", [128, 1, 256], BF16))
        b2_sb = stack.enter_context(nc.sbuf_tensor("b2_sb", [128, 1], F32))
        isrc_sb = stack.enter_context(
            nc.sbuf_tensor("isrc_sb", [128, c.idx_cols], I16))
        idst_sb = stack.enter_context(
            nc.sbuf_tensor("idst_sb", [128, c.idx_cols], I16))
        gAt = stack.enter_context(
            nc.sbuf_tensor("gAt", [128, 2, c.ngrp, 256], BF16))
        gBt = stack.enter_context(
            nc.sbuf_tensor("gBt", [128, 2, c.ngrp, 256], BF16))
        t1 = stack.enter_context(nc.sbuf_tensor("t1", [128, c.ngrp, 128], BF16))
        t2 = stack.enter_context(nc.sbuf_tensor("t2", [128, c.ngrp, 64], BF16))
        r_sb = stack.enter_context(nc.sbuf_tensor("r_sb", [128, c.ngrp], F32))
        rt = stack.enter_context(nc.sbuf_tensor("rt", [c.ngrp, 2, 128], F32))

        # raw semaphores (before Tile so Tile's sem range sits above)
        qsem = [stack.enter_context(nc.semaphore(f"qsem{i}"))  # noqa: ANT232
                for i in range(2 * NQ)]
        vfree = stack.enter_context(nc.semaphore("vfree"))
        vdone = stack.enter_context(nc.semaphore("vdone"))
        ofree = stack.enter_context(nc.semaphore("ofree"))

        a_tab = nc.dram_tensor("a_tab", [c.n_rows, 256], BF16)
        b_tab = nc.dram_tensor("b_tab", [c.n_rows, 256], BF16)

        with tile.TileContext(nc) as tc:
            with tc.tile_pool(name="const", bufs=1) as cpool:
                w1at_b = cpool.tile([128, 256], BF16, tag="w1at_b")
                w1bt_b = cpool.tile([128, 256], BF16, tag="w1bt_b")
                for name, dst in (("w1at", w1at_b), ("w1bt", w1bt_b)):
                    f = cpool.tile([128, 256], F32, tag=name + "_f")
                    nc.sync.dma_start(f[:], io[name][:])
                    nc.vector.tensor_copy(dst[:], f[:])
                b1_sb = cpool.tile([128, 256], F32, tag="b1_sb")
                nc.sync.dma_start(b1_sb[:], io["b1bc"][:])
                w2f = cpool.tile([128, 256], F32, tag="w2f")
                nc.sync.dma_start(w2f[:], io["w2bc"][:])
                nc.vector.tensor_copy(w2_b[:, 0, :], w2f[:])
                nc.sync.dma_start(b2_sb[:], io["b2bc"][:])
                nc.sync.dma_start(isrc_sb[:], io["isrc"][:])
                nc.sync.dma_start(idst_sb[:], io["idst"][:])

                # ---- Phase B ----
                nch = c.node_chunk
                spg = nch // 128
                with (
                    tc.tile_pool(name="xb", bufs=4) as xpool,
                    tc.tile_pool(name="tst", bufs=3) as spool,
                    tc.tile_pool(name="ps", bufs=6, space="PSUM") as pspool,
                ):
                    for xt, wb, tab, is_a in (
                        (io["xt_dis"], w1at_b, a_tab, True),
                        (io["xt_drug"], w1bt_b, b_tab, False),
                    ):
                        for ci in range(_cdiv(c.n_nodes, nch)):
                            c0 = ci * nch
                            cw = min(nch, c.n_nodes - c0)
                            xb = xpool.tile([128, nch], BF16, tag="xb")
                            # SWDGE cast-DMA f32->bf16
                            nc.gpsimd.dma_start(xb[:, :cw], xt[:, c0:c0 + cw])
                            st = spool.tile([128, spg, 256], BF16, tag="st")
                            full_g = cw // 128
                            rem = cw % 128
                            for g in range(_cdiv(cw, 128)):
                                sw = min(128, cw - g * 128)
                                ps = pspool.tile([128, 256], F32, tag="ps")
                                nc.tensor.matmul(
                                    out=ps[:sw, :],
                                    lhsT=xb[:, g * 128:g * 128 + sw],
                                    rhs=wb[:],
                                    start=True, stop=True,
                                )
                                if is_a:
                                    nc.vector.tensor_add(
                                        st[:sw, g, :], ps[:sw, :], b1_sb[:sw, :])
                                else:
                                    nc.scalar.copy(st[:sw, g, :], ps[:sw, :])
                            if full_g:
                                nc.sync.dma_start(
                                    tab[c0:c0 + full_g * 128, :].rearrange(
                                        "(g p) h -> p g h", p=128),
                                    st[:, :full_g, :],
                                )
                            if rem:
                                nc.sync.dma_start(
                                    tab[c0 + full_g * 128:
                                        c0 + full_g * 128 + rem, :],
                                    st[:rem, full_g, :],
                                )

                # -- Phase C (Tile-scheduled; gathers on 4 SWDGE queues) --
                wic = c.gi // 16
                self_tc = tc
                with (
                    self_tc.tile_pool(name="ga", bufs=2) as gapool,
                    self_tc.tile_pool(name="gb", bufs=2) as gbpool,
                    self_tc.tile_pool(name="o", bufs=2) as opool,
                ):
                        for b in range(c.nblk):
                            gA = gapool.tile([128, c.ngrp, 256], BF16, tag="gA")
                            gB = gbpool.tile([128, c.ngrp, 256], BF16, tag="gB")
                            for j in range(c.gpb):
                                col0 = (b * c.gpb + j) * wic
                                nc.gpsimd.dma_gather(
                                    gA[:, 8 * j:8 * j + 8, :],
                                    a_tab[:, :],
                                    isrc_sb[:, col0:col0 + wic],
                                    c.gi, c.gi, 256,
                                    queue_num=j % NQ,
                                )
                                nc.gpsimd.dma_gather(
                                    gB[:, 8 * j:8 * j + 8, :],
                                    b_tab[:, :],
                                    idst_sb[:, col0:col0 + wic],
                                    c.gi, c.gi, 256,
                                    queue_num=j % NQ,
                                )
                            nc.vector.tensor_add(gA[:], gA[:], gB[:])
                            nc.vector.scalar_tensor_tensor(
                                out=gA[:],
                                in0=gA[:],
                                scalar=0.0,
                                in1=w2_b[:].to_broadcast([128, c.ngrp, 256]),
                                op0=mybir.AluOpType.max,
                                op1=mybir.AluOpType.mult,
                            )
                            t1 = gbpool.tile([128, c.ngrp, 128], BF16, tag="t1")
                            nc.vector.tensor_add(t1[:], gA[:, :, 0:128], gA[:, :, 128:256])
                            t2 = gbpool.tile([128, c.ngrp, 64], BF16, tag="t2")
                            nc.vector.tensor_add(t2[:], t1[:, :, 0:64], t1[:, :, 64:128])
                            r = opool.tile([128, c.ngrp], F32, tag="r")
                            nc.vector.tensor_reduce(
                                out=r[:],
                                in_=t2[:],
                                axis=mybir.AxisListType.X,
                                op=mybir.AluOpType.add,
                            )
                            nc.vector.tensor_scalar_add(r[:], r[:], b2_sb[:, 0:1])
                            rto = opool.tile([c.ngrp, 128], F32, tag="rto")
                            for bi in range(4):
                                for bj in range(c.ngrp // 32):
                                    nc.vector.transpose(
                                        rto[bj * 32:bj * 32 + 32, bi * 32:bi * 32 + 32],
                                        r[bi * 32:bi * 32 + 32, bj * 32:bj * 32 + 32],
                                    )
                            nc.sync.dma_start(io["out"][b, :, :], rto[:, :])


# ---------------------------------------------------------------------------
# Host side
# ---------------------------------------------------------------------------

_CACHE = {}
last_result = None  # BassKernelResults of the most recent run


def _declare(nc, name, shape, dtype, is_out=False):
    return nc.declare_dram_parameter(name, list(shape), dtype, isOutput=is_out)


def _make_nc(cfg):
    nc = bacc.Bacc("TRN2", target_bir_lowering=False, debug=False,
                   num_devices=N_CORES, num_swdge_queues=NQ,
                   detect_race_conditions=False)
    io = {
        "xt_dis": _declare(nc, "xt_dis", [128, cfg.n_nodes], F32),
        "xt_drug": _declare(nc, "xt_drug", [128, cfg.n_nodes], F32),
        "w1at": _declare(nc, "w1at", [128, 256], F32),
        "w1bt": _declare(nc, "w1bt", [128, 256], F32),
        "b1bc": _declare(nc, "b1bc", [128, 256], F32),
        "w2bc": _declare(nc, "w2bc", [128, 256], F32),
        "b2bc": _declare(nc, "b2bc", [128, 1], F32),
        "isrc": _declare(nc, "isrc", [128, cfg.idx_cols], I16),
        "idst": _declare(nc, "idst", [128, cfg.idx_cols], I16),
        "out": _declare(nc, "out", [cfg.nblk, cfg.ngrp, 128], F32, is_out=True),
    }
    build(nc, io, cfg)
    nc.compile()
    return nc


def _get_nc_cached(cfg):
    key = (cfg.n_nodes, cfg.e_core)
    if key not in _CACHE:
        _CACHE[key] = _make_nc(cfg)
    return _CACHE[key]


def _install_ntff_hook():
    """Shim antenv.axon_hooks (absent in this image) so trace=True works."""
    import antenv
    if "antenv.axon_hooks" in sys.modules:
        return
    m = types.ModuleType("antenv.axon_hooks")
    m._hook = None
    m.set_axon_ntff_profile_hook = lambda h: setattr(m, "_hook", h)
    m.get_axon_ntff_profile_hook = lambda: m._hook
    sys.modules["antenv.axon_hooks"] = m
    antenv.axon_hooks = m
    try:
        from trn_agent_boot.trn_boot import _ntff_profile_via_ctypes
        m.set_axon_ntff_profile_hook(
            _ntff_profile_via_ctypes("/opt/axon/libaxon_pjrt.so"))
    except Exception:
        pass


def wrap_idx(idx_padded, cfg):
    """[e_pad] int16 -> [128, idx_cols] wrapped (16-row pattern x8)."""
    w = idx_padded.reshape(-1, 16).T  # logical i at [i%16, i//16]
    return np.ascontiguousarray(np.tile(w, (8, 1)))


def prep_in_maps(cfg, x_disease, x_drug, edge_label_index, W1, b1, W2, b2,
                 n_cores=N_CORES):
    hid = 2 * x_disease.shape[1]
    xt_dis = np.ascontiguousarray(x_disease.T, dtype=np.float32)
    xt_drug = np.ascontiguousarray(x_drug.T, dtype=np.float32)
    w1at = np.ascontiguousarray(W1[:, :128].T, dtype=np.float32)
    w1bt = np.ascontiguousarray(W1[:, 128:].T, dtype=np.float32)
    b1bc = np.ascontiguousarray(
        np.broadcast_to(b1.reshape(1, hid), (128, hid)), dtype=np.float32)
    w2bc = np.ascontiguousarray(
        np.broadcast_to(W2.reshape(1, hid), (128, hid)), dtype=np.float32)
    b2bc = np.full((128, 1), float(np.asarray(b2).reshape(-1)[0]), np.float32)

    e = np.asarray(edge_label_index)
    in_maps = []
    for core in range(n_cores):
        lo = core * cfg.e_core
        src = np.zeros(cfg.e_pad, np.int16)
        dst = np.zeros(cfg.e_pad, np.int16)
        src[:cfg.e_core] = e[0, lo:lo + cfg.e_core].astype(np.int16)
        dst[:cfg.e_core] = e[1, lo:lo + cfg.e_core].astype(np.int16)
        in_maps.append({
            "xt_dis": xt_dis, "xt_drug": xt_drug,
            "w1at": w1at, "w1bt": w1bt, "b1bc": b1bc,
            "w2bc": w2bc, "b2bc": b2bc,
            "isrc": wrap_idx(src, cfg),
            "idst": wrap_idx(dst, cfg),
        })
    return in_maps


def kernel(x_disease, x_drug, edge_label_index, W1, b1, W2, b2, _trace=False):
    global last_result
    from concourse.bass_utils import run_bass_kernel_spmd

    cfg = FULL
    if _trace:
        _install_ntff_hook()
    nc = _get_nc_cached(cfg)
    in_maps = prep_in_maps(cfg, x_disease, x_drug, edge_label_index,
                           W1, b1, W2, b2)
    res = run_bass_kernel_spmd(nc, in_maps, list(range(N_CORES)),
                               trace=_trace)
    last_result = res
    outs = [res.results[cr]["out"].reshape(-1)[:cfg.e_core]
            for cr in range(N_CORES)]
    return np.concatenate(outs).reshape(-1, 1).astype(np.float32)

